# revision 1
# baseline (speedup 1.0000x reference)
"""Bass/Trainium2 kernel for batched masked-Kabsch RMSD (nn_Coords2RMSD).

Strategy (per NeuronCore, SPMD across 8 cores):
  - Host sorts batch rows by num_atoms and forms 4 size classes (quartiles
    of the sorted order). Core c takes one 128-row tile from each class;
    class k is processed with a fixed atom capacity cap[k] (max natoms in
    the class, rounded up), so cores run identical programs while skipping
    the padded tail of short rows.
  - Per tile: SWDGE DMA cast-loads the f32 coords to bf16 SBUF, DVE builds
    the atom mask and de-interleaves xyz with the mask multiply, then 9
    scalar_tensor_tensor products with fused fp32 accumulation produce the
    raw second moments; ScalarE accumulates Sx/Sy/|x|^2/|y|^2.
  - Final stage (tiny [128, 4] fp32 tiles): centroid corrections, 3x3
    C^T C eigenvalues via the closed-form trigonometric method (acos built
    from Arctan+Sqrt, cos via Sin with phase bias), Kabsch det sign, RMSD.
"""

import numpy as np

import concourse.bass as bass
import concourse.mybir as mybir
from concourse.tile import TileContext, ScopedClock

F32 = mybir.dt.float32
BF16 = mybir.dt.bfloat16
OP = mybir.AluOpType
AF = mybir.ActivationFunctionType

N_CORES = 8
ROWS = 128  # rows per tile == SBUF partitions


# ---------------------------------------------------------------------------
# TileContext tail patch: this walrus build accepts at most ONE sync-wait
# command per instruction and no sem-eq waits, so the stock drain + EVSEM
# butterfly fails codegen. Emit a ge-wait-only tail instead.
# ---------------------------------------------------------------------------
def _patched_drain_and_barrier(self, tick_clock, wait_clock):
    nc = self.nc
    dummy = nc.gpsimd.nop()
    wait_clock.add_sem_waits(dummy.ins, ScopedClock({None: tick_clock.global_clock}))
    waits = list(dummy.ins.sync_info.on_wait) if dummy.ins.sync_info else []
    if dummy.ins.sync_info:
        dummy.ins.sync_info = mybir.SyncInfo(on_wait=[], on_update=[])

    bsem = nc.alloc_semaphore(f"tail_bsem_{nc.next_id()}")
    dsem = nc.alloc_semaphore(f"tail_dsem_{nc.next_id()}")
    n_eng = 0
    for eng in nc.engines.values():
        eng.drain()
        eng.sem_inc(bsem, 1)
        n_eng += 1
    # gpsimd observes every engine and every outstanding work/DMA sem, then
    # broadcasts that knowledge via dsem so the range-clear happens-after
    # everything on every engine.
    nc.gpsimd.wait_ge(bsem, n_eng)
    for w in waits:
        n = nc.gpsimd.nop()
        n.ins.sync_info = mybir.SyncInfo(on_wait=[w], on_update=[])
    nc.gpsimd.sem_inc(dsem, 1)
    for eng in nc.engines.values():
        if eng is not nc.gpsimd:
            eng.wait_ge(dsem, 1)

    popped = nc._tile_sem_poison_stack.pop()
    assert popped is self._sem_poison
    nc.clear_and_free_semaphores(list(self.sems.allocated().values()))
    nc.gpsimd.sem_clear(bsem)
    nc.gpsimd.sem_clear(dsem)


def install_tile_patch():
    TileContext._drain_and_barrier = _patched_drain_and_barrier


# ---------------------------------------------------------------------------
# BIR post-pass: this walrus build accepts at most one sync-wait command per
# instruction (none on Drain). Tile's sem-assigner can attach several, so
# split extras onto same-engine NoOps inserted just before the instruction.
# ---------------------------------------------------------------------------
_orig_to_json_bytes = bass.Bass.to_json_bytes


def _split_multiwait_json(self) -> bytes:
    import json

    raw = _orig_to_json_bytes(self)
    m = json.loads(raw)
    ctr = 0
    changed = False
    for f in m.get("functions", []):
        for blk in f.get("blocks", []):
            insts = blk.get("instructions", [])
            out = []
            for inst in insts:
                si = inst.get("sync_info")
                ow = (si or {}).get("on_wait") or []
                opc = str(inst.get("opcode", inst.get("type", "")))
                limit = 0 if opc == "Drain" else 1
                if len(ow) > limit:
                    keep = ow[len(ow) - limit :] if limit else []
                    moved = ow[: len(ow) - limit] if limit else ow
                    for w in moved:
                        ctr += 1
                        out.append(
                            {
                                "debug": inst.get("debug", 0),
                                "engine": inst["engine"],
                                "ins": [],
                                "name": f"WS-{ctr}-{inst['name']}",
                                "opcode": "NoOp",
                                "outs": [],
                                "sync_info": {"on_update": [], "on_wait": [w]},
                            }
                        )
                    si["on_wait"] = keep
                    changed = True
                out.append(inst)
            blk["instructions"] = out
    if not changed:
        return raw
    return json.dumps(m).encode()


bass.Bass.to_json_bytes = _split_multiwait_json


# ---------------------------------------------------------------------------
# Final math emitter: everything on [128, K] fp32 tiles.
# ---------------------------------------------------------------------------
class _FM:
    def __init__(self, nc, pool, K):
        self.nc = nc
        self.pool = pool
        self.K = K
        self.n = 0
        self._consts = {}

    def const_col(self, val):
        val = float(val)
        if val in self._consts:
            return self._consts[val]
        i = len(self._consts)
        t = self.pool.tile([ROWS, 1], F32, tag=f"fmc{i}", name=f"fmc{i}")
        self.nc.vector.memset(t[:], val)
        self._consts[val] = t[:]
        return t[:]

    def t(self):
        self.n += 1
        return self.pool.tile([ROWS, self.K], F32, tag=f"fm{self.n}", name=f"fm{self.n}")

    def tt(self, a, b, op):
        o = self.t()
        self.nc.vector.tensor_tensor(o[:], a, b, op)
        return o[:]

    def mul(self, a, b):
        return self.tt(a, b, OP.mult)

    def add(self, a, b):
        return self.tt(a, b, OP.add)

    def sub(self, a, b):
        return self.tt(a, b, OP.subtract)

    def ts(self, a, s, op):
        o = self.t()
        self.nc.vector.tensor_scalar(o[:], a, float(s), None, op)
        return o[:]

    def ts2(self, a, s1, s2, op0, op1):
        o = self.t()
        self.nc.vector.tensor_scalar(o[:], a, float(s1), float(s2), op0, op1)
        return o[:]

    def stt(self, a, s, b, op0, op1):
        """(a op0 s) op1 b"""
        o = self.t()
        self.nc.vector.scalar_tensor_tensor(o[:], a, float(s), b, op0, op1)
        return o[:]

    def act(self, a, func, bias=0.0, scale=1.0):
        o = self.t()
        if isinstance(bias, float) and bias not in (0.0, 1.0) and func != AF.Copy:
            bias = self.const_col(bias)
        self.nc.scalar.activation(o[:], a, func, bias=bias, scale=scale)
        return o[:]

    def recip(self, a):
        o = self.t()
        self.nc.vector.reciprocal(o[:], a)
        return o[:]


def _emit_final_math(nc, fm, st_rxy, st_g, st_s, meta_t, out_ap, K):
    n_ap = meta_t[:]
    rn = fm.recip(n_ap)

    def Sx(i):
        return st_s[:, i : 6 * K : 6]

    def Sy(j):
        return st_s[:, 3 + j : 6 * K : 6]

    def Rxy(i, j):
        return st_rxy[:, 3 * i + j : 9 * K : 9]

    # C_ij = Rxy_ij - Sx_i * Sy_j * rn
    C = [[None] * 3 for _ in range(3)]
    for i in range(3):
        for j in range(3):
            t = fm.mul(Sx(i), Sy(j))
            C[i][j] = fm.sub(Rxy(i, j), fm.mul(t, rn))

    # gx = Rxx - (|Sx|^2) rn ; gy = Ryy - (|Sy|^2) rn
    sx2 = fm.add(fm.add(fm.mul(Sx(0), Sx(0)), fm.mul(Sx(1), Sx(1))), fm.mul(Sx(2), Sx(2)))
    sy2 = fm.add(fm.add(fm.mul(Sy(0), Sy(0)), fm.mul(Sy(1), Sy(1))), fm.mul(Sy(2), Sy(2)))
    gx = fm.sub(st_g[:, 0 : 2 * K : 2], fm.mul(sx2, rn))
    gy = fm.sub(st_g[:, 1 : 2 * K : 2], fm.mul(sy2, rn))

    # K = C^T C (symmetric; k[a][b] = sum_i C[i][a] C[i][b])
    kk = {}
    for a in range(3):
        for b in range(a, 3):
            s = fm.mul(C[0][a], C[0][b])
            s = fm.add(s, fm.mul(C[1][a], C[1][b]))
            s = fm.add(s, fm.mul(C[2][a], C[2][b]))
            kk[(a, b)] = s

    # det(C)
    m0 = fm.sub(fm.mul(C[1][1], C[2][2]), fm.mul(C[1][2], C[2][1]))
    m1 = fm.sub(fm.mul(C[1][0], C[2][2]), fm.mul(C[1][2], C[2][0]))
    m2 = fm.sub(fm.mul(C[1][0], C[2][1]), fm.mul(C[1][1], C[2][0]))
    detC = fm.add(fm.sub(fm.mul(C[0][0], m0), fm.mul(C[0][1], m1)), fm.mul(C[0][2], m2))

    # q = tr(K)/3 ; p2 = sum (k_aa - q)^2 + 2 (k01^2 + k02^2 + k12^2)
    q = fm.ts(fm.add(fm.add(kk[(0, 0)], kk[(1, 1)]), kk[(2, 2)]), 1.0 / 3.0, OP.mult)
    kd = [fm.sub(kk[(a, a)], q) for a in range(3)]
    p2 = fm.add(fm.add(fm.mul(kd[0], kd[0]), fm.mul(kd[1], kd[1])), fm.mul(kd[2], kd[2]))
    xsq = fm.add(
        fm.add(fm.mul(kk[(0, 1)], kk[(0, 1)]), fm.mul(kk[(0, 2)], kk[(0, 2)])),
        fm.mul(kk[(1, 2)], kk[(1, 2)]),
    )
    p2 = fm.stt(xsq, 2.0, p2, OP.mult, OP.add)  # p2 + 2*xsq
    # p = sqrt(max(p2/6, tiny))
    p2c = fm.ts(fm.ts(p2, 1.0 / 6.0, OP.mult), 1e-30, OP.max)
    p = fm.act(p2c, AF.Sqrt)

    # det(K - qI) (symmetric)
    k01, k02, k12 = kk[(0, 1)], kk[(0, 2)], kk[(1, 2)]
    d0 = fm.mul(kd[0], fm.sub(fm.mul(kd[1], kd[2]), fm.mul(k12, k12)))
    d1 = fm.mul(k01, fm.sub(fm.mul(k01, kd[2]), fm.mul(k12, k02)))
    d2 = fm.mul(k02, fm.sub(fm.mul(k01, k12), fm.mul(kd[1], k02)))
    detKq = fm.add(fm.sub(d0, d1), d2)

    # r = 0.5 det(K-qI) / p^3, clamped to [-1, 1]
    rp = fm.recip(p)
    r = fm.mul(fm.mul(fm.ts(detKq, 0.5, OP.mult), rp), fm.mul(rp, rp))
    r = fm.ts(fm.ts(r, 1.0, OP.min), -1.0, OP.max)

    # acos via |r| fold (ScalarE Arctan only accepts [-pi/2, pi/2]):
    #   A = 2 atan(sqrt((1-|r|)/(1+|r|))) = acos(|r|), arg in [0, 1]
    #   acos(r) = A + (r<0) * (pi - 2A) ; phi = acos(r)/3
    rabs = fm.stt(r, -1.0, r, OP.mult, OP.max)  # |r| = max(-r, r)
    onemr = fm.act(rabs, AF.Identity, bias=1.0, scale=-1.0)  # 1 - |r|
    onepr = fm.ts(rabs, 1.0, OP.add)
    u = fm.mul(onemr, fm.recip(onepr))
    su = fm.act(u, AF.Sqrt)
    at = fm.act(su, AF.Arctan)
    A = fm.ts(at, 2.0, OP.mult)
    rneg = fm.ts(r, 0.0, OP.is_lt)
    corr = fm.ts2(A, -2.0, float(np.pi), OP.mult, OP.add)  # pi - 2A
    acr = fm.add(A, fm.mul(rneg, corr))
    # cos(phi) = sin(phi + pi/2), arg in [pi/2, pi/2+pi/3] ; 
    # cos(phi + 2pi/3) = -sin(5pi/6 - phi), arg in [pi/2, 5pi/6]
    c1 = fm.act(acr, AF.Sin, bias=float(np.pi / 2), scale=1.0 / 3.0)
    c3m = fm.act(acr, AF.Sin, bias=float(5 * np.pi / 6), scale=-1.0 / 3.0)

    # eigenvalues (l3 = q - 2p*c3m)
    p2x = fm.ts(p, 2.0, OP.mult)
    l1 = fm.add(q, fm.mul(p2x, c1))
    l3 = fm.sub(q, fm.mul(p2x, c3m))
    l2 = fm.sub(fm.stt(q, 3.0, l1, OP.mult, OP.subtract), l3)  # 3q - l1 - l3

    s1 = fm.act(fm.ts(l1, 0.0, OP.max), AF.Sqrt)
    s2 = fm.act(fm.ts(l2, 0.0, OP.max), AF.Sqrt)
    s3 = fm.act(fm.ts(l3, 0.0, OP.max), AF.Sqrt)

    # d = +1 if detC >= 0 else -1  ->  d = 1 - 2*(detC < 0)
    neg = fm.ts(detC, 0.0, OP.is_lt)
    d = fm.act(neg, AF.Identity, bias=1.0, scale=-2.0)

    tr = fm.add(fm.add(s1, s2), fm.mul(d, s3))

    # msd = (gx + gy - 2 tr) rn ; rmsd = sqrt(max(msd, 0))
    diff = fm.stt(tr, -2.0, fm.add(gx, gy), OP.mult, OP.add)
    msd = fm.mul(diff, rn)
    rmsd = fm.act(fm.ts(msd, 0.0, OP.max), AF.Sqrt)
    nc.vector.tensor_copy(out_ap, rmsd)


# ---------------------------------------------------------------------------
# Program builder
# ---------------------------------------------------------------------------
def build_program(caps, nmax, cfg=None):
    """caps: per-class atom capacities (len K). Returns nc."""
    cfg = cfg or {}
    cast_on_dma = cfg.get("cast_on_dma", True)
    dt_main = BF16 if cfg.get("bf16", True) else F32
    K = len(caps)
    capmax = max(caps)
    ncols = 3 * nmax

    install_tile_patch()
    nc = bass.Bass()
    x_d = nc.dram_tensor("x", [K * ROWS, ncols], F32, kind="ExternalInput")
    y_d = nc.dram_tensor("y", [K * ROWS, ncols], F32, kind="ExternalInput")
    iota_d = nc.dram_tensor("iota", [ROWS, nmax], F32, kind="ExternalInput")
    meta_d = nc.dram_tensor("meta", [ROWS, K], F32, kind="ExternalInput")
    out_d = nc.dram_tensor("out", [ROWS, K], F32, kind="ExternalOutput")

    with TileContext(nc) as tc:
        with (
            tc.tile_pool(name="const", bufs=1) as constp,
            tc.tile_pool(name="raw", bufs=cfg.get("raw_bufs", 2)) as rawp,
            tc.tile_pool(name="masked", bufs=cfg.get("masked_bufs", 2)) as mp,
            tc.tile_pool(name="scratch", bufs=1) as scrp,
            tc.tile_pool(name="stats", bufs=1) as statp,
        ):
            iota_t = constp.tile([ROWS, nmax], F32)
            nc.sync.dma_start(out=iota_t[:], in_=iota_d[:])
            meta_t = constp.tile([ROWS, K], F32)
            nc.sync.dma_start(out=meta_t[:], in_=meta_d[:])

            st_rxy = statp.tile([ROWS, 9 * K], F32)
            st_g = statp.tile([ROWS, 2 * K], F32)
            st_s = statp.tile([ROWS, 6 * K], F32)

            for t, cap in enumerate(caps):
                W = 3 * cap
                x_raw = rawp.tile([ROWS, W], dt_main if cast_on_dma else F32, tag="x_raw")
                y_raw = rawp.tile([ROWS, W], dt_main if cast_on_dma else F32, tag="y_raw")
                dma_eng = nc.gpsimd if cast_on_dma else nc.sync
                # rows are component-major on the host side: [x0..xN y0..yN z0..zN]
                x_src = x_d[t * ROWS : (t + 1) * ROWS, :].rearrange(
                    "p (c n) -> p c n", c=3
                )[:, :, 0:cap]
                y_src = y_d[t * ROWS : (t + 1) * ROWS, :].rearrange(
                    "p (c n) -> p c n", c=3
                )[:, :, 0:cap]
                dma_eng.dma_start(out=x_raw[:].rearrange("p (c n) -> p c n", c=3), in_=x_src)
                dma_eng.dma_start(out=y_raw[:].rearrange("p (c n) -> p c n", c=3), in_=y_src)

                m_t = mp.tile([ROWS, cap], dt_main, tag="mask")
                nc.vector.tensor_scalar(
                    m_t[:], iota_t[:, 0:cap], meta_t[:, t : t + 1], None, OP.is_lt
                )

                xm = mp.tile([ROWS, W], dt_main, tag="xm")
                ym = mp.tile([ROWS, W], dt_main, tag="ym")
                for i in range(3):
                    sl = slice(i * cap, (i + 1) * cap)
                    nc.vector.tensor_tensor(xm[:, sl], x_raw[:, sl], m_t[:], OP.mult)
                    nc.vector.tensor_tensor(ym[:, sl], y_raw[:, sl], m_t[:], OP.mult)

                ps = scrp.tile([ROWS, capmax], dt_main, tag="prod")
                for i in range(3):
                    for j in range(3):
                        col = 9 * t + 3 * i + j
                        nc.vector.scalar_tensor_tensor(
                            ps[:, 0:cap],
                            xm[:, i * cap : (i + 1) * cap],
                            1.0,
                            ym[:, j * cap : (j + 1) * cap],
                            OP.mult,
                            OP.mult,
                            accum_out=st_rxy[:, col : col + 1],
                        )

                sq = scrp.tile([ROWS, 3 * capmax], dt_main, tag="sq")
                nc.scalar.activation(
                    sq[:, 0:W], xm[:], AF.Square, accum_out=st_g[:, 2 * t : 2 * t + 1]
                )
                nc.scalar.activation(
                    sq[:, 0:W], ym[:], AF.Square, accum_out=st_g[:, 2 * t + 1 : 2 * t + 2]
                )
                cp = scrp.tile([ROWS, capmax], dt_main, tag="cp")
                for i in range(3):
                    nc.scalar.activation(
                        cp[:, 0:cap],
                        xm[:, i * cap : (i + 1) * cap],
                        AF.Identity,
                        accum_out=st_s[:, 6 * t + i : 6 * t + i + 1],
                    )
                for j in range(3):
                    nc.scalar.activation(
                        cp[:, 0:cap],
                        ym[:, j * cap : (j + 1) * cap],
                        AF.Identity,
                        accum_out=st_s[:, 6 * t + 3 + j : 6 * t + 4 + j],
                    )

            out_t = statp.tile([ROWS, K], F32)
            fm = _FM(nc, statp, K)
            _emit_final_math(nc, fm, st_rxy, st_g, st_s, meta_t, out_t[:], K)
            nc.sync.dma_start(out=out_d[:], in_=out_t[:])

    return nc


# ---------------------------------------------------------------------------
# Host side
# ---------------------------------------------------------------------------
def plan_shards(num_atoms, n_classes=4, cap_round=16):
    B = num_atoms.shape[0]
    assert B % (N_CORES * ROWS) == 0
    n_classes_total = B // (N_CORES * ROWS)
    assert n_classes == n_classes_total
    order = np.argsort(num_atoms, kind="stable")
    na_sorted = num_atoms[order]
    rows_per_class = N_CORES * ROWS
    caps = []
    for k in range(n_classes):
        mx = int(na_sorted[(k + 1) * rows_per_class - 1])
        cap = ((mx + cap_round - 1) // cap_round) * cap_round
        caps.append(cap)
    return order, caps


def shard_inputs(coords_input, coords_target, num_atoms, order, caps, nmax):
    K = len(caps)
    rows_per_class = N_CORES * ROWS
    iota = np.ascontiguousarray(
        np.broadcast_to(np.arange(nmax, dtype=np.float32), (ROWS, nmax))
    )
    in_maps = []
    core_row_idx = []
    for c in range(N_CORES):
        idx = np.concatenate(
            [
                order[k * rows_per_class + c * ROWS : k * rows_per_class + (c + 1) * ROWS]
                for k in range(K)
            ]
        )
        core_row_idx.append(idx)
        nmax_l = coords_input.shape[1] // 3
        xs = np.ascontiguousarray(
            coords_input[idx].reshape(-1, nmax_l, 3).transpose(0, 2, 1).reshape(len(idx), -1)
        )
        ys = np.ascontiguousarray(
            coords_target[idx].reshape(-1, nmax_l, 3).transpose(0, 2, 1).reshape(len(idx), -1)
        )
        meta = np.ascontiguousarray(
            num_atoms[idx].astype(np.float32).reshape(K, ROWS).T
        )
        in_maps.append({"x": xs, "y": ys, "iota": iota, "meta": meta})
    return in_maps, core_row_idx


def unshard_outputs(results, core_row_idx, B):
    out = np.empty(B, dtype=np.float32)
    K = results[0]["out"].shape[1]
    for c in range(N_CORES):
        o = results[c]["out"]  # [ROWS, K]
        idx = core_row_idx[c]
        out[idx] = o.T.reshape(-1)
    return out


# ---------------------------------------------------------------------------
# Entry point: full inputs in, full output out. Shards across 8 NeuronCores.
# ---------------------------------------------------------------------------
_PROG_CACHE = {}


def _get_program(caps, nmax):
    key = (tuple(caps), nmax)
    if key not in _PROG_CACHE:
        _PROG_CACHE[key] = build_program(list(caps), nmax)
    return _PROG_CACHE[key]


def kernel(coords_input, coords_target, num_atoms):
    from concourse.bass_utils import run_bass_kernel_spmd

    x = np.ascontiguousarray(np.asarray(coords_input, dtype=np.float32))
    y = np.ascontiguousarray(np.asarray(coords_target, dtype=np.float32))
    na = np.asarray(num_atoms)
    na_i = na.astype(np.int64)
    B, ncols = x.shape
    nmax = ncols // 3
    K = B // (N_CORES * ROWS)
    assert B == N_CORES * ROWS * K, f"unsupported batch {B}"

    order, caps = plan_shards(na_i, n_classes=K)
    in_maps, core_row_idx = shard_inputs(x, y, na_i, order, caps, nmax)
    nc = _get_program(caps, nmax)
    res = run_bass_kernel_spmd(nc, in_maps, core_ids=list(range(N_CORES)))
    out = unshard_outputs(res.results, core_row_idx, B)
    return out.astype(np.float32)



# revision 5
# speedup vs baseline: 2.1035x; 2.1035x over previous
"""Bass/Trainium2 kernel for batched masked-Kabsch RMSD (nn_Coords2RMSD).

Strategy (per NeuronCore, SPMD across 8 cores):
  - Host sorts the 4096 rows by num_atoms into 128 global groups of 32 rows
    (4 lanes x 8 cores). Per group, atom capacity is rounded to 128-atom
    chunks; rows are zero-padded to the group cap (masking happens on host).
  - Host packs, per core, an atom-major bf16 tensor z[128, TC]: for each
    (group q, chunk k) a 25-column block [x lanes b=0..3 (3 comps each) |
    y lanes | ones]. The PE engine computes the Gram matrix Z^T Z per group,
    accumulated over chunks in PSUM: one 25x25 Gram holds the 3x3
    cross-covariance C per lane, Gxx/Gyy second moments, and Sx/Sy sums
    (via the ones column) -- all reductions over atoms in one matmul stream.
  - Extraction: PSUM -> SBUF copy (ScalarE), then PE transposes rearrange
    the per-group Grams into a [128 groups, 625] stats tile (row-major per
    (c,p)); strided APs then address each quantity per lane.
  - Final stage on [128, 4] fp32 tiles: centroid corrections, 3x3 C^T C
    eigenvalues via the trigonometric method, Kabsch det sign, RMSD.
"""

import numpy as np
import ml_dtypes

import concourse.bass as bass
import concourse.mybir as mybir
from concourse.tile import TileContext, ScopedClock
from concourse.masks import make_identity

F32 = mybir.dt.float32
BF16 = mybir.dt.bfloat16
OP = mybir.AluOpType
AF = mybir.ActivationFunctionType

N_CORES = 8
GROUPS = 128          # global groups == stats partition dim
LANES = 4             # rows per group per core
GROUP_ROWS = LANES * N_CORES  # 32 sorted rows per group
CHUNK = 128           # atoms per matmul chunk (contraction partitions)
ZCOLS = 6 * LANES + 1  # 25: x(12) | y(12) | ones
NBLK = 4              # process groups in 4 blocks of 32
BLK = GROUPS // NBLK


# ---------------------------------------------------------------------------
# TileContext tail patch: this walrus build accepts at most ONE sync-wait
# command per instruction and no sem-eq waits, so the stock drain + EVSEM
# butterfly fails codegen. Emit a ge-wait-only tail instead.
# ---------------------------------------------------------------------------
def _patched_drain_and_barrier(self, tick_clock, wait_clock):
    nc = self.nc
    dummy = nc.gpsimd.nop()
    wait_clock.add_sem_waits(dummy.ins, ScopedClock({None: tick_clock.global_clock}))
    waits = list(dummy.ins.sync_info.on_wait) if dummy.ins.sync_info else []
    if dummy.ins.sync_info:
        dummy.ins.sync_info = mybir.SyncInfo(on_wait=[], on_update=[])

    bsem = nc.alloc_semaphore(f"tail_bsem_{nc.next_id()}")
    dsem = nc.alloc_semaphore(f"tail_dsem_{nc.next_id()}")
    n_eng = 0
    for eng in nc.engines.values():
        eng.drain()
        eng.sem_inc(bsem, 1)
        n_eng += 1
    nc.gpsimd.wait_ge(bsem, n_eng)
    for w in waits:
        n = nc.gpsimd.nop()
        n.ins.sync_info = mybir.SyncInfo(on_wait=[w], on_update=[])
    nc.gpsimd.sem_inc(dsem, 1)
    for eng in nc.engines.values():
        if eng is not nc.gpsimd:
            eng.wait_ge(dsem, 1)

    popped = nc._tile_sem_poison_stack.pop()
    assert popped is self._sem_poison
    nc.clear_and_free_semaphores(list(self.sems.allocated().values()))
    nc.gpsimd.sem_clear(bsem)
    nc.gpsimd.sem_clear(dsem)


def install_tile_patch():
    TileContext._drain_and_barrier = _patched_drain_and_barrier


# ---------------------------------------------------------------------------
# BIR post-pass: this walrus build accepts at most one sync-wait command per
# instruction (none on Drain). Tile's sem-assigner can attach several, so
# split extras onto same-engine NoOps inserted just before the instruction.
# ---------------------------------------------------------------------------
_orig_to_json_bytes = bass.Bass.to_json_bytes


def _split_multiwait_json(self) -> bytes:
    import json

    raw = _orig_to_json_bytes(self)
    m = json.loads(raw)
    ctr = 0
    changed = False
    for f in m.get("functions", []):
        for blk in f.get("blocks", []):
            insts = blk.get("instructions", [])
            out = []
            for inst in insts:
                si = inst.get("sync_info")
                ow = (si or {}).get("on_wait") or []
                opc = str(inst.get("opcode", inst.get("type", "")))
                limit = 0 if opc == "Drain" else 1
                if len(ow) > limit:
                    keep = ow[len(ow) - limit :] if limit else []
                    moved = ow[: len(ow) - limit] if limit else ow
                    for w in moved:
                        ctr += 1
                        out.append(
                            {
                                "debug": inst.get("debug", 0),
                                "engine": inst["engine"],
                                "ins": [],
                                "name": f"WS-{ctr}-{inst['name']}",
                                "opcode": "NoOp",
                                "outs": [],
                                "sync_info": {"on_update": [], "on_wait": [w]},
                            }
                        )
                    si["on_wait"] = keep
                    changed = True
                out.append(inst)
            blk["instructions"] = out
    if not changed:
        return raw
    return json.dumps(m).encode()


bass.Bass.to_json_bytes = _split_multiwait_json


# ---------------------------------------------------------------------------
# Final math emitter: everything on [128, K] fp32 tiles (K = LANES).
# ---------------------------------------------------------------------------
class _FM:
    def __init__(self, nc, pool, K):
        self.nc = nc
        self.pool = pool
        self.K = K
        self.n = 0
        self._consts = {}

    def const_col(self, val):
        val = float(val)
        if val in self._consts:
            return self._consts[val]
        i = len(self._consts)
        t = self.pool.tile([GROUPS, 1], F32, tag=f"fmc{i}", name=f"fmc{i}")
        self.nc.vector.memset(t[:], val)
        self._consts[val] = t[:]
        return t[:]

    def t(self):
        self.n += 1
        return self.pool.tile([GROUPS, self.K], F32, tag=f"fm{self.n}", name=f"fm{self.n}")

    def tt(self, a, b, op):
        o = self.t()
        self.nc.vector.tensor_tensor(o[:], a, b, op)
        return o[:]

    def mul(self, a, b):
        return self.tt(a, b, OP.mult)

    def add(self, a, b):
        return self.tt(a, b, OP.add)

    def sub(self, a, b):
        return self.tt(a, b, OP.subtract)

    def ts(self, a, s, op):
        o = self.t()
        self.nc.vector.tensor_scalar(o[:], a, float(s), None, op)
        return o[:]

    def ts2(self, a, s1, s2, op0, op1):
        o = self.t()
        self.nc.vector.tensor_scalar(o[:], a, float(s1), float(s2), op0, op1)
        return o[:]

    def stt(self, a, s, b, op0, op1):
        """(a op0 s) op1 b"""
        o = self.t()
        self.nc.vector.scalar_tensor_tensor(o[:], a, float(s), b, op0, op1)
        return o[:]

    def act(self, a, func, bias=0.0, scale=1.0):
        o = self.t()
        if isinstance(bias, float) and bias not in (0.0, 1.0) and func != AF.Copy:
            bias = self.const_col(bias)
        self.nc.scalar.activation(o[:], a, func, bias=bias, scale=scale)
        return o[:]

    def recip(self, a):
        o = self.t()
        self.nc.vector.reciprocal(o[:], a)
        return o[:]


def _emit_final_math(nc, fm, Rxy, Sx, Sy, gxr_ap, gyr_ap, n_ap, out_ap):
    rn = fm.recip(n_ap)

    # C_ij = Rxy_ij - Sx_i * Sy_j * rn
    C = [[None] * 3 for _ in range(3)]
    for i in range(3):
        for j in range(3):
            t = fm.mul(Sx(i), Sy(j))
            C[i][j] = fm.sub(Rxy(i, j), fm.mul(t, rn))

    # gx = Gxx_trace - |Sx|^2 rn ; gy likewise
    sx2 = fm.add(fm.add(fm.mul(Sx(0), Sx(0)), fm.mul(Sx(1), Sx(1))), fm.mul(Sx(2), Sx(2)))
    sy2 = fm.add(fm.add(fm.mul(Sy(0), Sy(0)), fm.mul(Sy(1), Sy(1))), fm.mul(Sy(2), Sy(2)))
    gx = fm.sub(gxr_ap, fm.mul(sx2, rn))
    gy = fm.sub(gyr_ap, fm.mul(sy2, rn))

    # K = C^T C (symmetric; k[a][b] = sum_i C[i][a] C[i][b])
    kk = {}
    for a in range(3):
        for b in range(a, 3):
            s = fm.mul(C[0][a], C[0][b])
            s = fm.add(s, fm.mul(C[1][a], C[1][b]))
            s = fm.add(s, fm.mul(C[2][a], C[2][b]))
            kk[(a, b)] = s

    # det(C)
    m0 = fm.sub(fm.mul(C[1][1], C[2][2]), fm.mul(C[1][2], C[2][1]))
    m1 = fm.sub(fm.mul(C[1][0], C[2][2]), fm.mul(C[1][2], C[2][0]))
    m2 = fm.sub(fm.mul(C[1][0], C[2][1]), fm.mul(C[1][1], C[2][0]))
    detC = fm.add(fm.sub(fm.mul(C[0][0], m0), fm.mul(C[0][1], m1)), fm.mul(C[0][2], m2))

    # q = tr(K)/3 ; p2 = sum (k_aa - q)^2 + 2 (k01^2 + k02^2 + k12^2)
    q = fm.ts(fm.add(fm.add(kk[(0, 0)], kk[(1, 1)]), kk[(2, 2)]), 1.0 / 3.0, OP.mult)
    kd = [fm.sub(kk[(a, a)], q) for a in range(3)]
    p2 = fm.add(fm.add(fm.mul(kd[0], kd[0]), fm.mul(kd[1], kd[1])), fm.mul(kd[2], kd[2]))
    xsq = fm.add(
        fm.add(fm.mul(kk[(0, 1)], kk[(0, 1)]), fm.mul(kk[(0, 2)], kk[(0, 2)])),
        fm.mul(kk[(1, 2)], kk[(1, 2)]),
    )
    p2 = fm.stt(xsq, 2.0, p2, OP.mult, OP.add)  # p2 + 2*xsq
    p2c = fm.ts(fm.ts(p2, 1.0 / 6.0, OP.mult), 1e-30, OP.max)
    p = fm.act(p2c, AF.Sqrt)

    # det(K - qI) (symmetric)
    k01, k02, k12 = kk[(0, 1)], kk[(0, 2)], kk[(1, 2)]
    d0 = fm.mul(kd[0], fm.sub(fm.mul(kd[1], kd[2]), fm.mul(k12, k12)))
    d1 = fm.mul(k01, fm.sub(fm.mul(k01, kd[2]), fm.mul(k12, k02)))
    d2 = fm.mul(k02, fm.sub(fm.mul(k01, k12), fm.mul(kd[1], k02)))
    detKq = fm.add(fm.sub(d0, d1), d2)

    # r = 0.5 det(K-qI) / p^3, clamped to [-1, 1]
    rp = fm.recip(p)
    r = fm.mul(fm.mul(fm.ts(detKq, 0.5, OP.mult), rp), fm.mul(rp, rp))
    r = fm.ts(fm.ts(r, 1.0, OP.min), -1.0, OP.max)

    # acos via |r| fold (ScalarE Arctan only accepts [-pi/2, pi/2]):
    #   A = 2 atan(sqrt((1-|r|)/(1+|r|))) = acos(|r|), arg in [0, 1]
    #   acos(r) = A + (r<0) * (pi - 2A) ; phi = acos(r)/3
    rabs = fm.stt(r, -1.0, r, OP.mult, OP.max)  # |r| = max(-r, r)
    onemr = fm.ts2(rabs, -1.0, 1.0, OP.mult, OP.add)  # 1 - |r|
    onepr = fm.ts(rabs, 1.0, OP.add)
    u = fm.mul(onemr, fm.recip(onepr))
    su = fm.act(u, AF.Sqrt)
    at = fm.act(su, AF.Arctan)
    A = fm.ts(at, 2.0, OP.mult)
    rneg = fm.ts(r, 0.0, OP.is_lt)
    corr = fm.ts2(A, -2.0, float(np.pi), OP.mult, OP.add)  # pi - 2A
    acr = fm.add(A, fm.mul(rneg, corr))
    # cos(phi) = sin(phi + pi/2) ; cos(phi + 2pi/3) = -sin(5pi/6 - phi)
    c1 = fm.act(acr, AF.Sin, bias=float(np.pi / 2), scale=1.0 / 3.0)
    c3m = fm.act(acr, AF.Sin, bias=float(5 * np.pi / 6), scale=-1.0 / 3.0)

    # eigenvalues (l3 = q - 2p*c3m)
    p2x = fm.ts(p, 2.0, OP.mult)
    l1 = fm.add(q, fm.mul(p2x, c1))
    l3 = fm.sub(q, fm.mul(p2x, c3m))
    l2 = fm.sub(fm.stt(q, 3.0, l1, OP.mult, OP.subtract), l3)  # 3q - l1 - l3

    s1 = fm.act(fm.ts(l1, 0.0, OP.max), AF.Sqrt)
    s2 = fm.act(fm.ts(l2, 0.0, OP.max), AF.Sqrt)
    s3 = fm.act(fm.ts(l3, 0.0, OP.max), AF.Sqrt)

    # d = +1 if detC >= 0 else -1  ->  d = 1 - 2*(detC < 0)
    neg = fm.ts(detC, 0.0, OP.is_lt)
    d = fm.ts2(neg, -2.0, 1.0, OP.mult, OP.add)

    tr = fm.add(fm.add(s1, s2), fm.mul(d, s3))

    # msd = (gx + gy - 2 tr) rn ; rmsd = sqrt(max(msd, 0))
    diff = fm.stt(tr, -2.0, fm.add(gx, gy), OP.mult, OP.add)
    msd = fm.mul(diff, rn)
    rmsd = fm.act(fm.ts(msd, 0.0, OP.max), AF.Sqrt)
    nc.vector.tensor_copy(out_ap, rmsd)


# ---------------------------------------------------------------------------
# Program builder. chunks: per-group chunk counts (len 128, same on all cores).
# ---------------------------------------------------------------------------
def build_program(chunks):
    chunks = list(chunks)
    assert len(chunks) == GROUPS
    colstart = np.concatenate([[0], np.cumsum(np.asarray(chunks) * ZCOLS)]).astype(int)
    TC = int(colstart[-1])
    PP = ZCOLS * ZCOLS  # 625 stats cols per group

    install_tile_patch()
    nc = bass.Bass()
    z_d = nc.dram_tensor("z", [CHUNK, TC], BF16, kind="ExternalInput")
    meta_d = nc.dram_tensor("meta", [GROUPS, LANES], F32, kind="ExternalInput")
    out_d = nc.dram_tensor("out", [GROUPS, LANES], F32, kind="ExternalOutput")

    with TileContext(nc) as tc:
        with (
            tc.tile_pool(name="const", bufs=1) as constp,
            tc.tile_pool(name="z", bufs=1) as zp,
            tc.tile_pool(name="pcopy", bufs=2) as pcp,
            tc.tile_pool(name="stats", bufs=1) as statp,
            tc.tile_pool(name="psum1", bufs=2, space="PSUM") as ps1p,
            tc.tile_pool(name="psum2", bufs=1, space="PSUM") as ps2p,
        ):
            meta_t = constp.tile([GROUPS, LANES], F32)
            nc.sync.dma_start(out=meta_t[:], in_=meta_d[:])
            ident = constp.tile([ZCOLS, ZCOLS], F32)
            make_identity(nc, ident[:])
            # Pre-load the Sqrt activation table while DMAs stream so the
            # final stage pays only the trig<->sqrt switches.
            scr = constp.tile([GROUPS, 1], F32)
            nc.vector.memset(scr[:], 1.0)
            nc.scalar.activation(scr[:], scr[:], AF.Sqrt)

            # Input tiles: one per half-block (16 groups), loaded by one DMA
            # each; matmuls for those groups depend only on their own tile.
            HB = GROUPS // 16  # 8 half-blocks
            zt = []
            for h in range(HB):
                c0 = int(colstart[h * 16])
                c1 = int(colstart[(h + 1) * 16])
                t = zp.tile([CHUNK, c1 - c0], BF16, tag=f"z{h}", name=f"z{h}")
                nc.sync.dma_start(out=t[:], in_=z_d[:, c0:c1])
                zt.append((t, c0))

            # Gram matmuls: per block of 32 groups, two PSUM tiles (16 groups
            # each, one bank apiece); accumulate over chunks.
            P_all = pcp.tile([ZCOLS, GROUPS * ZCOLS], F32, tag="Pall")
            for j in range(NBLK):
                p1 = [
                    ps1p.tile([ZCOLS, 16 * ZCOLS], F32, tag=f"p1a", name=f"p1a{j}"),
                    ps1p.tile([ZCOLS, 16 * ZCOLS], F32, tag=f"p1b", name=f"p1b{j}"),
                ]
                for g in range(BLK):
                    q = j * BLK + g
                    t, c0 = zt[q // 16]
                    tgt = p1[(g // 16)][:, ZCOLS * (g % 16) : ZCOLS * (g % 16 + 1)]
                    nchunks = chunks[q]
                    for k in range(nchunks):
                        o = int(colstart[q]) - c0 + k * ZCOLS
                        zs = t[:, o : o + ZCOLS]
                        nc.tensor.matmul(
                            tgt, zs, zs, start=(k == 0), stop=(k == nchunks - 1)
                        )
                # PSUM -> SBUF so the next block can reuse the banks.
                o = j * BLK * ZCOLS
                nc.scalar.activation(P_all[:, o : o + 16 * ZCOLS], p1[0][:], AF.Identity)
                nc.scalar.activation(
                    P_all[:, o + 16 * ZCOLS : o + BLK * ZCOLS], p1[1][:], AF.Identity
                )

            # Transpose the per-group Grams into [group, 25c+p] layout: one
            # transpose per Gram column c covers all 128 groups.
            ps2a = ps2p.tile([GROUPS, 16 * ZCOLS], F32, tag="ps2a")
            ps2b = ps2p.tile([GROUPS, 9 * ZCOLS], F32, tag="ps2b")
            for c in range(ZCOLS):
                src = P_all[:, c : GROUPS * ZCOLS : ZCOLS]  # [25, 128]
                if c < 16:
                    dst = ps2a[:, ZCOLS * c : ZCOLS * (c + 1)]
                else:
                    dst = ps2b[:, ZCOLS * (c - 16) : ZCOLS * (c - 15)]
                nc.tensor.transpose(dst, src, ident[:])

            stats = statp.tile([GROUPS, PP], F32)
            nc.scalar.activation(stats[:, 0 : 16 * ZCOLS], ps2a[:], AF.Identity)
            nc.scalar.activation(stats[:, 16 * ZCOLS :], ps2b[:], AF.Identity)

            # stats[q, 25*c + p] = Gram[p, c]; cols per lane b:
            #   x rows p = 3b+i, y rows p = 12+3b+j, ones p/c = 24
            def Rxy(i, j):  # Gram[3b+i, 12+3b+j]
                base = 25 * (12 + j) + i  # + 78*b
                return stats[:, base : base + 78 * 3 + 1 : 78]

            def Sx(i):  # Gram[3b+i, 24]
                base = 25 * 24 + i
                return stats[:, base : base + 3 * 3 + 1 : 3]

            def Sy(j):  # Gram[12+3b+j, 24]
                base = 25 * 24 + 12 + j
                return stats[:, base : base + 3 * 3 + 1 : 3]

            fm = _FM(nc, statp, LANES)

            def diag_sum(off):
                # Gram[off+3b+i, off+3b+i] -> col = 25*(off)+off + 78b + 26i
                a = [
                    stats[:, 26 * off + 26 * i : 26 * off + 26 * i + 78 * 3 + 1 : 78]
                    for i in range(3)
                ]
                return fm.add(fm.add(a[0], a[1]), a[2])

            gxr = diag_sum(0)
            gyr = diag_sum(12)

            out_t = statp.tile([GROUPS, LANES], F32)
            _emit_final_math(nc, fm, Rxy, Sx, Sy, gxr, gyr, meta_t[:], out_t[:])
            nc.sync.dma_start(out=out_d[:], in_=out_t[:])

    return nc


# ---------------------------------------------------------------------------
# Host side
# ---------------------------------------------------------------------------
def plan_shards(num_atoms):
    na = np.asarray(num_atoms).astype(np.int64)
    B = na.shape[0]
    assert B == GROUPS * GROUP_ROWS, f"unsupported batch {B}"
    order = np.argsort(na, kind="stable")
    caps = na[order].reshape(GROUPS, GROUP_ROWS)[:, -1]
    chunks = np.maximum(1, -(-caps // CHUNK)).astype(int)  # ceil
    return order, chunks


def shard_inputs(coords_input, coords_target, num_atoms, order, chunks):
    B, f = coords_input.shape
    nmax = f // 3
    na = np.asarray(num_atoms).astype(np.int64)
    x3 = coords_input.reshape(B, nmax, 3)
    y3 = coords_target.reshape(B, nmax, 3)
    colstart = np.concatenate([[0], np.cumsum(chunks * ZCOLS)]).astype(int)
    TC = int(colstart[-1])

    in_maps = []
    for c in range(N_CORES):
        z = np.zeros((CHUNK, TC), dtype=ml_dtypes.bfloat16)
        meta = np.empty((GROUPS, LANES), np.float32)
        for v in np.unique(chunks):
            qs = np.where(chunks == v)[0]
            nq = len(qs)
            A = int(v) * CHUNK
            # rows for (q, b): order[q*32 + b*8 + c]
            ridx = order[(qs[:, None] * GROUP_ROWS) + np.arange(LANES)[None, :] * N_CORES + c]
            nar = na[ridx]  # [nq, LANES]
            meta[qs, :] = nar.astype(np.float32)
            mask = (np.arange(A)[None, None, :] < nar[:, :, None]).astype(np.float32)
            xa = x3[ridx.ravel(), :A, :].reshape(nq, LANES, A, 3) * mask[..., None]
            ya = y3[ridx.ravel(), :A, :].reshape(nq, LANES, A, 3) * mask[..., None]
            # [nq, LANES, v, 128, 3] -> [nq, v, 128, LANES*3]
            xt = xa.reshape(nq, LANES, int(v), CHUNK, 3).transpose(0, 2, 3, 1, 4)
            yt = ya.reshape(nq, LANES, int(v), CHUNK, 3).transpose(0, 2, 3, 1, 4)
            buf = np.empty((nq, int(v), CHUNK, ZCOLS), np.float32)
            buf[..., 0 : 3 * LANES] = xt.reshape(nq, int(v), CHUNK, 3 * LANES)
            buf[..., 3 * LANES : 6 * LANES] = yt.reshape(nq, int(v), CHUNK, 3 * LANES)
            buf[..., 6 * LANES] = 1.0
            colidx = (
                colstart[qs][:, None] + np.arange(int(v) * ZCOLS)[None, :]
            ).ravel()
            z[:, colidx] = (
                buf.transpose(2, 0, 1, 3).reshape(CHUNK, nq * int(v) * ZCOLS)
            ).astype(ml_dtypes.bfloat16)
        in_maps.append({"z": z, "meta": meta})
    return in_maps


def unshard_outputs(results, order, B):
    out = np.empty(B, dtype=np.float32)
    for c in range(N_CORES):
        o = np.asarray(results[c]["out"], np.float32)  # [GROUPS, LANES]
        q = np.arange(GROUPS)[:, None]
        b = np.arange(LANES)[None, :]
        rows = order[q * GROUP_ROWS + b * N_CORES + c]
        out[rows] = o
    return out


# ---------------------------------------------------------------------------
# Entry point: full inputs in, full output out. Shards across 8 NeuronCores.
# ---------------------------------------------------------------------------
_PROG_CACHE = {}


def _get_program(chunks):
    key = tuple(int(v) for v in chunks)
    if key not in _PROG_CACHE:
        _PROG_CACHE[key] = build_program(list(key))
    return _PROG_CACHE[key]


def kernel(coords_input, coords_target, num_atoms):
    from concourse.bass_utils import run_bass_kernel_spmd

    x = np.ascontiguousarray(np.asarray(coords_input, dtype=np.float32))
    y = np.ascontiguousarray(np.asarray(coords_target, dtype=np.float32))
    na = np.asarray(num_atoms).astype(np.int64)
    B = x.shape[0]

    order, chunks = plan_shards(na)
    in_maps = shard_inputs(x, y, na, order, chunks)
    nc = _get_program(chunks)
    res = run_bass_kernel_spmd(nc, in_maps, core_ids=list(range(N_CORES)))
    return unshard_outputs(res.results, order, B).astype(np.float32)
